# revision 16
# baseline (speedup 1.0000x reference)
"""Trainium2 Bass kernel for nn_ExpertAdaRMSLayer (AdaRMS transformer layer).

Sharding: 8 cores = 4 batches (DP) x 2 token-halves. Each core computes its
1024 tokens end-to-end with no collectives; k/v (nkv=1) are computed
redundantly by the pair of cores sharing a batch. All activations are kept
feature-major [feature, token] on device; the host pre-transposes inputs /
weights and re-assembles the output. Columns are rolled per core so "own"
tokens are always columns 0..1023 (keeps the SPMD program uniform; attention
is permutation-invariant over keys).

Perf structure (v2):
- AdaRMS gains g = w*(1 + t@tw.T) are computed on the HOST and folded into
  the columns of Wq/Wk/Wv (g1) and Wg/Wu (g2) per batch, so the device only
  does h = x * rsqrt(mean(x^2)+eps).
- rsqrt / softmax reciprocals use the fast custom-DVE reciprocal; the
  per-token scale row is broadcast to 128 partitions with a K=1 ones-matmul
  into PSUM (no DRAM bounce).
- rms1 is computed in 4 token slices software-pipelined with the Q
  projection; attention heads are software-pipelined (colsum(h),
  scores(h+1), bcast(h), ctx(h)) so the scalar-engine exp never stalls PE;
  o_proj+rms2 are split per token slice so the MLP starts early.
- Weight strips stream on the sync-engine DMA queue only (prefetch);
  activations use the gpsimd queue.
"""

import os
import sys
from contextlib import ExitStack

import numpy as np

sys.path.insert(0, "/opt/trn_rl_repo")

import ml_dtypes

import concourse.bass as bass
import concourse.mybir as mybir
import concourse.tile as tile

BF16 = ml_dtypes.bfloat16
F32 = np.float32

# Model dims (hardcoded per spec)
HIDDEN, NQ, NKV, HD, INTER = 2048, 8, 1, 256, 8192
B, S = 4, 2048
EPS = 1e-6
ROPE_BASE = 10000.0

P = 128
HC = HIDDEN // P          # 16 hidden chunks
IC = INTER // P           # 64 inter chunks
QC = (NQ * HD) // P       # 16 q-feature chunks
KC_HD = HD // P           # 2 head-dim chunks
T_OWN = S // 2            # 1024 own tokens per core
T_FULL = S                # 2048 tokens per batch
FD = 512                  # matmul free-dim tile (one PSUM bank of f32)
NT_OWN = T_OWN // FD      # 2
NT_FULL = T_FULL // FD    # 4
N_CORES = 8

DT = mybir.dt.bfloat16    # matmul operand dtype
DT_R = mybir.dt.float32r  # full-rate fp32 dtype for rms sum-of-squares
AF = mybir.ActivationFunctionType
ALU = mybir.AluOpType

_CACHE = {}
LAST_RESULTS = None


PADW = 132  # padded strip row length: keeps strip DMAs off the 1-wait
             # DIRECT2D path (3 unmergeable src dims -> generic DMA)


def _strips(WT, KC, MC):
    """WT: [K, M] f32 with rows = contraction dim. Returns bf16 array
    [MC, 128, KC, PADW] with [m][p][kc][:128] = WT[kc*128+p, m*128+j]."""
    K, M = WT.shape
    assert K == KC * P and M == MC * P
    A = WT.reshape(KC, P, MC, P).transpose(2, 1, 0, 3)
    out = np.zeros((MC, P, KC, PADW), dtype=BF16)
    out[:, :, :, :P] = A.astype(BF16)
    return out


def build_program():
    if "nc" in _CACHE:
        return _CACHE["nc"]

    nc = bass.Bass()
    dram = {}

    def inp(name, shape, dt):
        dram[name] = nc.declare_dram_parameter(name, list(shape), dt,
                                               isOutput=False)

    inp("xT", (HIDDEN, T_FULL), DT)
    inp("cosT", (P, T_FULL), DT)
    inp("sinT", (P, T_FULL), DT)
    inp("wq", (QC, P, HC, PADW), DT)
    inp("wk", (KC_HD, P, HC, PADW), DT)
    inp("wv", (HC, P, HD), DT)
    inp("wo", (HC, P, QC, PADW), DT)
    inp("wg", (IC, P, HC, PADW), DT)
    inp("wu", (IC, P, HC, PADW), DT)
    inp("wd", (HC, P, IC, PADW), DT)
    outT = nc.declare_dram_parameter("outT", [HIDDEN, T_OWN],
                                     mybir.dt.float32, isOutput=True)
    if os.environ.get("KERNEL_DEBUG_DUMP"):
        for nm, shp in (("dbg_h1T", [HIDDEN, T_FULL]), ("dbg_qT", [NQ * HD, T_OWN]),
                        ("dbg_kT", [HD, T_FULL]), ("dbg_v", [T_FULL, HD]),
                        ("dbg_ctxT", [NQ * HD, T_OWN])):
            dram[nm] = nc.dram_tensor(nm, shp, mybir.dt.float32)
    res2T = nc.dram_tensor("res2T", [HIDDEN, T_OWN], mybir.dt.float32)

    _build_kernel(nc, dram, outT, res2T)
    if not os.environ.get("KERNEL_NO_WAIT_SPLIT"):
        _split_dma_waits(nc)
    _CACHE["nc"] = nc
    return nc


def _split_dma_waits(nc):
    """This walrus encodes at most ONE sync-wait per instruction (the ISA
    EVENTS struct has a single wait slot and this build refuses to split).
    Hoist all waits of multi-wait instructions onto standalone
    event-semaphore instructions on the issuing engine/sequencer, which
    executes them in program order before the original instruction."""
    n = 0
    for f in nc.m.functions:
        for bb in f.blocks:
            out = []
            changed = False
            for inst in bb.instructions:
                si = inst.sync_info
                if si is not None and len(si.on_wait) > 1:
                    for w in si.on_wait:
                        ev = mybir.InstEventSemaphore(
                            name=f"{inst.name}_w{n}", ins=[], outs=[])
                        ev.engine = inst.engine
                        ev.sync_info = mybir.SyncInfo(on_wait=[w],
                                                      on_update=[])
                        out.append(ev)
                        n += 1
                    inst.sync_info = mybir.SyncInfo(
                        on_wait=[], on_update=list(si.on_update))
                    changed = True
                out.append(inst)
            if changed:
                bb.instructions[:] = out
    return n


def _build_kernel(nc, dram, outT, res2T):
    xT_v = dram["xT"][:, :].rearrange("(c p) t -> p c t", p=P)
    res2T_v = res2T[:, :].rearrange("(c p) t -> p c t", p=P)
    outT_v = outT[:, :].rearrange("(c p) t -> p c t", p=P)

    with tile.TileContext(nc) as tc, ExitStack() as top:
        const = top.enter_context(tc.tile_pool(name="const", bufs=1))
        psA = top.enter_context(tc.tile_pool(name="psA", bufs=5, space="PSUM"))
        psB = top.enter_context(tc.tile_pool(name="psB", bufs=2, space="PSUM"))
        psR = top.enter_context(tc.tile_pool(name="psR", bufs=1, space="PSUM"))

        ones_bf = const.tile([P, 1], DT)
        nc.vector.memset(ones_bf, 1.0)
        ones_rf = const.tile([P, 1], mybir.dt.float32, name="ones_rf")
        nc.vector.memset(ones_rf, 1.0)
        ones_r = ones_rf.bitcast(DT_R)
        onesrow_f = const.tile([1, P], mybir.dt.float32, name="onesrow_f")
        nc.vector.memset(onesrow_f, 1.0)
        onesrow_r = onesrow_f.bitcast(DT_R)
        onesrow_bf = const.tile([1, P], DT, name="onesrow_bf")
        nc.vector.memset(onesrow_bf, 1.0)
        strips = top.enter_context(tc.tile_pool(name="strips", bufs=4))

        # ---------------- stage B+C: ada_rms1 (4 slices) pipelined with QKV
        poolBC = tc.alloc_tile_pool(name="poolBC", bufs=1)
        h1T = poolBC.tile([P, HC, T_FULL], DT, name="h1T")
        poolCD = tc.alloc_tile_pool(name="poolCD", bufs=1, side="right")
        qT = poolCD.tile([P, QC, T_OWN], DT, name="qT")
        kT = poolCD.tile([P, KC_HD, T_FULL], DT, name="kT")
        vtok = poolCD.tile([P, HC, HD], DT, name="vtok")

        with tc.tile_pool(name="stBC", bufs=2) as sC:
            cos_f = sC.tile([P, T_FULL], DT, tag="cos", bufs=1, name="cos_f")
            sin_f = sC.tile([P, T_FULL], DT, tag="sin", bufs=1, name="sin_f")
            nc.gpsimd.dma_start(out=cos_f, in_=dram["cosT"][:, :])
            nc.gpsimd.dma_start(out=sin_f, in_=dram["sinT"][:, :])

            rr_tiles = {}

            def rms1_pre(s):
                """load slice s, squares, col sum-of-squares, rsqrt -> rr."""
                sl = slice(s * FD, (s + 1) * FD)
                xt = sC.tile([P, HC, FD], DT, tag="xt", name="xt", bufs=2)
                eng = nc.sync if s < 2 else nc.gpsimd
                eng.dma_start(out=xt, in_=xT_v[:, :, sl])
                ssum = psB.tile([1, FD], mybir.dt.float32, tag="psmall",
                                name="ps_ss")
                for kc in range(HC):
                    sq = sC.tile([P, FD], DT_R, tag="sq", name="sq", bufs=2)
                    nc.scalar.activation(sq, xt[:, kc, :], AF.Square)
                    nc.tensor.matmul(ssum, ones_r, sq,
                                     start=(kc == 0), stop=(kc == HC - 1))
                inv = sC.tile([1, FD], mybir.dt.float32, tag="inv",
                              name="inv", bufs=2)
                nc.vector.reciprocal(inv, ssum)
                # rsqrt(mean_sq) = sqrt(H / ssum); eps is negligible here.
                rr = sC.tile([1, FD], DT_R, tag="rr", name="rr", bufs=2)
                nc.scalar.activation(rr, inv, AF.Sqrt, scale=float(HIDDEN))
                rr_tiles[s] = (xt, rr)

            def rms1_post(s):
                """broadcast rr to all partitions, h1T slice = x * rr."""
                sl = slice(s * FD, (s + 1) * FD)
                xt, rr = rr_tiles.pop(s)
                rrb = psR.tile([P, FD], mybir.dt.float32, tag="bc",
                               name="ps_rrb")
                nc.tensor.matmul(rrb, onesrow_r, rr, start=True, stop=True)
                for kc in range(HC):
                    nc.vector.tensor_mul(h1T[:, kc, sl], xt[:, kc, :], rrb)

            def qproj(nt):
                sl = slice(nt * FD, (nt + 1) * FD)
                for m in range(QC):
                    strip = strips.tile([P, HC, P], DT, tag="w",
                                        name="wq_strip")
                    nc.sync.dma_start(out=strip, in_=dram["wq"][m][:, :, :P])
                    ps = psA.tile([P, FD], mybir.dt.float32, tag="pmm",
                                  name="ps_q")
                    for kc in range(HC):
                        nc.tensor.matmul(ps, strip[:, kc, :], h1T[:, kc, sl],
                                         start=(kc == 0), stop=(kc == HC - 1))
                    nc.scalar.copy(out=qT[:, m, sl], in_=ps)

            rms1_pre(0)
            rms1_pre(1)
            rms1_post(0)
            rms1_post(1)
            rms1_pre(2)
            rms1_pre(3)
            qproj(0)
            rms1_post(2)
            rms1_post(3)
            qproj(1)

            # K projection over the full batch
            for m in range(KC_HD):
                strip = strips.tile([P, HC, P], DT, tag="w",
                                    name="wk_strip")
                nc.sync.dma_start(out=strip, in_=dram["wk"][m][:, :, :P])
                for nt in range(NT_FULL):
                    sl = slice(nt * FD, (nt + 1) * FD)
                    ps = psA.tile([P, FD], mybir.dt.float32, tag="pmm",
                                  name="ps_k")
                    for kc in range(HC):
                        nc.tensor.matmul(ps, strip[:, kc, :], h1T[:, kc, sl],
                                         start=(kc == 0), stop=(kc == HC - 1))
                    nc.scalar.copy(out=kT[:, m, sl], in_=ps)

            def rope_pair(u, v_, cos_t, sin_t, width, eng, tg):
                t1 = sC.tile([P, width], DT, tag=tg + "1", bufs=1, name="rp1")
                t2 = sC.tile([P, width], DT, tag=tg + "2", bufs=1, name="rp2")
                t3 = sC.tile([P, width], DT, tag=tg + "3", bufs=1, name="rp3")
                t4 = sC.tile([P, width], DT, tag=tg + "4", bufs=1, name="rp4")
                eng.tensor_mul(t1, u, cos_t)
                eng.tensor_mul(t2, u, sin_t)
                eng.tensor_mul(t3, v_, sin_t)
                eng.tensor_mul(t4, v_, cos_t)
                eng.tensor_sub(u, t1, t3)
                eng.tensor_add(v_, t4, t2)

            for hh in range(2):
                ksl = slice(hh * T_OWN, (hh + 1) * T_OWN)
                rope_pair(kT[:, 0, ksl], kT[:, 1, ksl], cos_f[:, ksl],
                          sin_f[:, ksl], T_OWN, nc.gpsimd, "rtg")

            # v token-major: [key-token-in-chunk, key-chunk, hd]
            wv_sb = sC.tile([P, HC, HD], DT, tag="wv", bufs=1, name="wv_sb")
            nc.sync.dma_start(out=wv_sb, in_=dram["wv"][:, :, :].rearrange("c p d -> p c d"))
            for tm in range(T_FULL // P):
                ps = psA.tile([P, HD], mybir.dt.float32, tag="pmm",
                              name="ps_v")
                tsl = slice(tm * P, (tm + 1) * P)
                for kc in range(HC):
                    nc.tensor.matmul(ps, h1T[:, kc, tsl], wv_sb[:, kc, :],
                                     start=(kc == 0), stop=(kc == HC - 1))
                nc.scalar.copy(out=vtok[:, tm, :], in_=ps)

            for h in range(NQ):
                eng, tg = (nc.vector, "rtv") if h < 4 else (nc.gpsimd, "rtg")
                rope_pair(qT[:, 2 * h, :], qT[:, 2 * h + 1, :],
                          cos_f[:, :T_OWN], sin_f[:, :T_OWN], T_OWN, eng, tg)

            if "dbg_h1T" in dram:
                dv = dram["dbg_h1T"][:, :].rearrange("(c p) t -> p c t", p=P)
                nc.gpsimd.dma_start(out=dv, in_=h1T)
                dv = dram["dbg_qT"][:, :].rearrange("(c p) t -> p c t", p=P)
                nc.gpsimd.dma_start(out=dv, in_=qT)
                dv = dram["dbg_kT"][:, :].rearrange("(c p) t -> p c t", p=P)
                nc.gpsimd.dma_start(out=dv, in_=kT)
                dv = dram["dbg_v"][:, :].rearrange("(c p) t -> p c t", p=P)
                nc.gpsimd.dma_start(out=dv, in_=vtok)
        poolBC.release()

        # ---------------- stage D: attention, software-pipelined heads
        poolDE = tc.alloc_tile_pool(name="poolDE", bufs=1)
        ctxT = poolDE.tile([P, QC, T_OWN], DT, name="ctxT")
        with tc.tile_pool(name="stD", bufs=2) as sD:

            def scores(h, at):
                for nt in range(NT_OWN):
                    sl = slice(nt * FD, (nt + 1) * FD)
                    for sm in range(T_FULL // P):
                        ps = psA.tile([P, FD], mybir.dt.float32, tag="pmm",
                                      name="ps_sc")
                        for dc in range(KC_HD):
                            nc.tensor.matmul(
                                ps, kT[:, dc, sm * P:(sm + 1) * P],
                                qT[:, 2 * h + dc, sl],
                                start=(dc == 0), stop=(dc == KC_HD - 1))
                        nc.scalar.activation(at[:, sm, sl], ps, AF.Exp,
                                             scale=1.0 / 16.0)

            def colsum_recip(h, at, recs):
                for nt in range(NT_OWN):
                    sl = slice(nt * FD, (nt + 1) * FD)
                    cs = psB.tile([1, FD], mybir.dt.float32, tag="psmall",
                                  name="ps_cs")
                    for kc in range(HC):
                        nc.tensor.matmul(cs, ones_bf, at[:, kc, sl],
                                         start=(kc == 0), stop=(kc == HC - 1))
                    rec = sD.tile([1, FD], mybir.dt.float32, tag=f"rec{nt}",
                                  bufs=2, name="rec")
                    nc.vector.reciprocal(rec, cs)
                    rec_bf = sD.tile([1, FD], DT, tag=f"recbf{nt}",
                                     bufs=2, name="rec_bf")
                    nc.vector.tensor_scalar_add(out=rec_bf, in0=rec,
                                                scalar1=0.0)
                    recs.append(rec_bf)

            def bcast_recb(h, recs, recb):
                for nt in range(NT_OWN):
                    sl = slice(nt * FD, (nt + 1) * FD)
                    rb = psR.tile([P, FD], mybir.dt.float32, tag="bc",
                                  name="ps_recb")
                    nc.tensor.matmul(rb, onesrow_bf, recs[nt],
                                     start=True, stop=True)
                    nc.vector.tensor_scalar_add(out=recb[:, sl], in0=rb,
                                                scalar1=0.0)

            def ctx(h, at, recb):
                for dm in range(KC_HD):
                    for nt in range(NT_OWN):
                        sl = slice(nt * FD, (nt + 1) * FD)
                        ps = psA.tile([P, FD], mybir.dt.float32, tag="pmm",
                                      name="ps_ctx")
                        for kc in range(HC):
                            nc.tensor.matmul(
                                ps, vtok[:, kc, dm * P:(dm + 1) * P],
                                at[:, kc, sl],
                                start=(kc == 0), stop=(kc == HC - 1))
                        nc.vector.tensor_mul(ctxT[:, 2 * h + dm, sl], ps,
                                             recb[:, sl])

            at_tiles = {}
            recb_tiles = {}
            rec_lists = {}

            def new_head(h):
                at_tiles[h] = sD.tile([P, HC, T_OWN], DT, tag="attn", bufs=2,
                                      name="attnT")
                scores(h, at_tiles[h])

            new_head(0)
            for h in range(NQ):
                rec_lists[h] = []
                colsum_recip(h, at_tiles[h], rec_lists[h])
                if h + 1 < NQ:
                    new_head(h + 1)
                recb_tiles[h] = sD.tile([P, T_OWN], DT, tag="recb", bufs=2,
                                        name="recb")
                bcast_recb(h, rec_lists[h], recb_tiles[h])
                ctx(h, at_tiles[h], recb_tiles[h])
                del at_tiles[h], recb_tiles[h], rec_lists[h]

            if "dbg_ctxT" in dram:
                dv = dram["dbg_ctxT"][:, :].rearrange("(c p) t -> p c t", p=P)
                nc.gpsimd.dma_start(out=dv, in_=ctxT)
        poolCD.release()

        # ---------------- stage E: o_proj + residual + ada_rms2, per slice
        poolEF = tc.alloc_tile_pool(name="poolEF", bufs=1, side="right")
        h2T = poolEF.tile([P, HC, T_OWN], DT, name="h2T")
        with tc.tile_pool(name="stE", bufs=2) as sE:
            for nt in range(NT_OWN):
                sl = slice(nt * FD, (nt + 1) * FD)
                res2 = sE.tile([P, HC, FD], mybir.dt.float32, tag="res2",
                               bufs=2, name="res2")
                for m in range(HC):
                    strip = strips.tile([P, QC, P], DT, tag="w",
                                        name="wo_strip")
                    nc.sync.dma_start(out=strip, in_=dram["wo"][m][:, :, :P])
                    ps = psA.tile([P, FD], mybir.dt.float32, tag="pmm",
                                  name="ps_o")
                    for kc in range(QC):
                        nc.tensor.matmul(ps, strip[:, kc, :], ctxT[:, kc, sl],
                                         start=(kc == 0), stop=(kc == QC - 1))
                    xo = sE.tile([P, FD], DT, tag="xo", bufs=4, name="xo")
                    nc.gpsimd.dma_start(out=xo, in_=xT_v[:, m, sl])
                    nc.vector.scalar_tensor_tensor(
                        out=res2[:, m, :], in0=ps, scalar=0.0,
                        in1=xo, op0=ALU.bypass, op1=ALU.add)
                ssum = psB.tile([1, FD], mybir.dt.float32, tag="psmall",
                                name="ps_ss2")
                for kc in range(HC):
                    sq = sE.tile([P, FD], DT_R, tag="sq", name="sq2", bufs=2)
                    nc.scalar.activation(sq, res2[:, kc, :], AF.Square)
                    nc.tensor.matmul(ssum, ones_r, sq,
                                     start=(kc == 0), stop=(kc == HC - 1))
                inv = sE.tile([1, FD], mybir.dt.float32, tag="inv",
                              name="inv2", bufs=2)
                nc.vector.reciprocal(inv, ssum)
                rr = sE.tile([1, FD], DT_R, tag="rr", name="rr2", bufs=2)
                nc.scalar.activation(rr, inv, AF.Sqrt, scale=float(HIDDEN))
                rrb = psR.tile([P, FD], mybir.dt.float32, tag="bc",
                               name="ps_rrb2")
                nc.tensor.matmul(rrb, onesrow_r, rr, start=True, stop=True)
                for kc in range(HC):
                    nc.vector.tensor_mul(h2T[:, kc, sl], res2[:, kc, :], rrb)
                nc.gpsimd.dma_start(out=res2T_v[:, :, sl], in_=res2)
        poolDE.release()

        # ---------------- stage F: SwiGLU MLP + final residual
        with tc.tile_pool(name="stF", bufs=2) as sF:
            for tt in range(NT_OWN):
                sl = slice(tt * FD, (tt + 1) * FD)
                act = sF.tile([P, IC, FD], DT, tag="act", bufs=1, name="act")
                for im in range(IC):
                    gstrip = strips.tile([P, HC, P], DT, tag="w",
                                         name="wg_strip")
                    nc.sync.dma_start(out=gstrip, in_=dram["wg"][im][:, :, :P])
                    ps_g = psA.tile([P, FD], mybir.dt.float32, tag="pmm",
                                    name="ps_g")
                    for kc in range(HC):
                        nc.tensor.matmul(ps_g, gstrip[:, kc, :],
                                         h2T[:, kc, sl],
                                         start=(kc == 0), stop=(kc == HC - 1))
                    sil = sF.tile([P, FD], mybir.dt.float32, tag="sil",
                                  name="sil")
                    nc.scalar.activation(sil, ps_g, AF.Sigmoid)
                    nc.vector.tensor_mul(sil, sil, ps_g)
                    ustrip = strips.tile([P, HC, P], DT, tag="w",
                                          name="wu_strip")
                    nc.sync.dma_start(out=ustrip, in_=dram["wu"][im][:, :, :P])
                    ps_u = psA.tile([P, FD], mybir.dt.float32, tag="pmm",
                                    name="ps_u")
                    for kc in range(HC):
                        nc.tensor.matmul(ps_u, ustrip[:, kc, :],
                                         h2T[:, kc, sl],
                                         start=(kc == 0), stop=(kc == HC - 1))
                    nc.vector.tensor_mul(act[:, im, :], sil, ps_u)
                for dm in range(HC):
                    dstrip = sF.tile([P, IC, P], DT, tag="wd", bufs=3,
                                     name="wd_strip")
                    nc.sync.dma_start(out=dstrip, in_=dram["wd"][dm][:, :, :P])
                    ps_d = psA.tile([P, FD], mybir.dt.float32, tag="pmm",
                                    name="ps_d")
                    for kc in range(IC):
                        nc.tensor.matmul(ps_d, dstrip[:, kc, :],
                                         act[:, kc, :],
                                         start=(kc == 0), stop=(kc == IC - 1))
                    r2c = sF.tile([P, FD], mybir.dt.float32, tag="r2c",
                                  name="r2c", bufs=4)
                    nc.gpsimd.dma_start(out=r2c, in_=res2T_v[:, dm, sl])
                    ot = sF.tile([P, FD], mybir.dt.float32, tag="ot",
                                 name="ot", bufs=4)
                    nc.vector.tensor_add(ot, ps_d, r2c)
                    nc.scalar.dma_start(out=outT_v[:, dm, sl], in_=ot)
        poolEF.release()


def _prep_inputs(x, pos_ids, time_emb, ln1_w, ln1_tw, ln2_w, ln2_tw,
                 Wq, Wk, Wv, Wo, Wg, Wu, Wd):
    """Host-side layout prep. Gains g = w*(1+t@tw.T) are folded into the
    input columns of Wq/Wk/Wv (g1) and Wg/Wu (g2), per batch.
    Returns list of per-core in_maps."""
    x = np.asarray(x, dtype=np.float32)
    time_emb = np.asarray(time_emb, dtype=np.float32)
    g1 = np.asarray(ln1_w) * (1.0 + time_emb @ np.asarray(ln1_tw).T)  # (B,H)
    g2 = np.asarray(ln2_w) * (1.0 + time_emb @ np.asarray(ln2_tw).T)  # (B,H)

    shared = {
        "wo": _strips(np.asarray(Wo).T, QC, HC),
        "wd": _strips(np.asarray(Wd).T, IC, HC),
    }
    per_batch = []
    for b in range(B):
        wq = _strips((np.asarray(Wq) * g1[b][None, :]).T, HC, QC)
        wk = _strips((np.asarray(Wk) * g1[b][None, :]).T, HC, KC_HD)
        wv = np.ascontiguousarray(
            (np.asarray(Wv) * g1[b][None, :]).T.reshape(HC, P, HD)
        ).astype(BF16)
        wg = _strips((np.asarray(Wg) * g2[b][None, :]).T, HC, IC)
        wu = _strips((np.asarray(Wu) * g2[b][None, :]).T, HC, IC)
        per_batch.append({"wq": wq, "wk": wk, "wv": wv, "wg": wg, "wu": wu})

    inv_freq = 1.0 / (ROPE_BASE **
                      (np.arange(0, HD, 2, dtype=np.float64) / HD))
    in_maps = []
    for c in range(N_CORES):
        b, half = c // 2, c % 2
        perm = np.r_[np.arange(half * T_OWN, (half + 1) * T_OWN),
                     np.arange((1 - half) * T_OWN, (2 - half) * T_OWN)]
        xTb = np.ascontiguousarray(np.asarray(x[b]).T[:, perm]).astype(BF16)
        ang = (np.asarray(pos_ids[b])[perm].astype(np.float64)[:, None]
               * inv_freq[None, :])
        m = dict(shared)
        m.update(per_batch[b])
        m["xT"] = xTb
        m["cosT"] = np.ascontiguousarray(np.cos(ang).T).astype(BF16)
        m["sinT"] = np.ascontiguousarray(np.sin(ang).T).astype(BF16)
        in_maps.append(m)
    return in_maps


def kernel(**inputs):
    global LAST_RESULTS
    from concourse.bass_utils import run_bass_kernel_spmd

    nc = build_program()
    in_maps = _prep_inputs(**{k: np.asarray(v) for k, v in inputs.items()})
    trace = bool(int(os.environ.get("KERNEL_TRACE", "0")))
    kw = {}
    if os.environ.get("KERNEL_TMPDIR"):
        os.makedirs(os.environ["KERNEL_TMPDIR"], exist_ok=True)
        kw["tmpdir"] = os.environ["KERNEL_TMPDIR"]
    res = run_bass_kernel_spmd(nc, in_maps, core_ids=list(range(N_CORES)),
                               trace=trace, **kw)
    LAST_RESULTS = res
    out = np.empty((B, S, HIDDEN), dtype=F32)
    for c in range(N_CORES):
        b, half = c // 2, c % 2
        out[b, half * T_OWN:(half + 1) * T_OWN, :] = res.results[c]["outT"].T
    return out


# revision 17
# speedup vs baseline: 1.0614x; 1.0614x over previous
"""Trainium2 Bass kernel for nn_ExpertAdaRMSLayer (AdaRMS transformer layer).

Sharding: 8 cores = 4 batches (DP) x 2 token-halves. Each core computes its
1024 tokens end-to-end with no collectives; k/v (nkv=1) are computed
redundantly by the pair of cores sharing a batch. All activations are kept
feature-major [feature, token] on device; the host pre-transposes inputs /
weights and re-assembles the output. Columns are rolled per core so "own"
tokens are always columns 0..1023 (keeps the SPMD program uniform; attention
is permutation-invariant over keys).

Perf structure (v2):
- AdaRMS gains g = w*(1 + t@tw.T) are computed on the HOST and folded into
  the columns of Wq/Wk/Wv (g1) and Wg/Wu (g2) per batch, so the device only
  does h = x * rsqrt(mean(x^2)+eps).
- rsqrt / softmax reciprocals use the fast custom-DVE reciprocal; the
  per-token scale row is broadcast to 128 partitions with a K=1 ones-matmul
  into PSUM (no DRAM bounce).
- rms1 is computed in 4 token slices software-pipelined with the Q
  projection; attention heads are software-pipelined (colsum(h),
  scores(h+1), bcast(h), ctx(h)) so the scalar-engine exp never stalls PE;
  o_proj+rms2 are split per token slice so the MLP starts early.
- Weight strips stream on the sync-engine DMA queue only (prefetch);
  activations use the gpsimd queue.
"""

import os
import sys
from contextlib import ExitStack

import numpy as np

sys.path.insert(0, "/opt/trn_rl_repo")

import ml_dtypes

import concourse.bass as bass
import concourse.mybir as mybir
import concourse.tile as tile

BF16 = ml_dtypes.bfloat16
F32 = np.float32

# Model dims (hardcoded per spec)
HIDDEN, NQ, NKV, HD, INTER = 2048, 8, 1, 256, 8192
B, S = 4, 2048
EPS = 1e-6
ROPE_BASE = 10000.0

P = 128
HC = HIDDEN // P          # 16 hidden chunks
IC = INTER // P           # 64 inter chunks
QC = (NQ * HD) // P       # 16 q-feature chunks
KC_HD = HD // P           # 2 head-dim chunks
T_OWN = S // 2            # 1024 own tokens per core
T_FULL = S                # 2048 tokens per batch
FD = 512                  # matmul free-dim tile (one PSUM bank of f32)
NT_OWN = T_OWN // FD      # 2
NT_FULL = T_FULL // FD    # 4
N_CORES = 8

DT = mybir.dt.bfloat16    # matmul operand dtype
DT_R = mybir.dt.float32r  # full-rate fp32 dtype for rms sum-of-squares
AF = mybir.ActivationFunctionType
ALU = mybir.AluOpType

_CACHE = {}
LAST_RESULTS = None


PADW = 132  # padded strip row length: keeps strip DMAs off the 1-wait
             # DIRECT2D path (3 unmergeable src dims -> generic DMA)


def _strips(WT, KC, MC):
    """WT: [K, M] f32 with rows = contraction dim. Returns bf16 array
    [MC, 128, KC, PADW] with [m][p][kc][:128] = WT[kc*128+p, m*128+j]."""
    K, M = WT.shape
    assert K == KC * P and M == MC * P
    A = WT.reshape(KC, P, MC, P).transpose(2, 1, 0, 3)
    out = np.zeros((MC, P, KC, PADW), dtype=BF16)
    out[:, :, :, :P] = A.astype(BF16)
    return out


def build_program():
    if "nc" in _CACHE:
        return _CACHE["nc"]

    nc = bass.Bass()
    dram = {}

    def inp(name, shape, dt):
        dram[name] = nc.declare_dram_parameter(name, list(shape), dt,
                                               isOutput=False)

    inp("xT", (HIDDEN, T_FULL), DT)
    inp("cosT", (P, T_FULL), DT)
    inp("sinT", (P, T_FULL), DT)
    inp("wq", (QC, P, HC, PADW), DT)
    inp("wk", (KC_HD, P, HC, PADW), DT)
    inp("wv", (HC, P, HD), DT)
    inp("wo", (HC, P, QC, PADW), DT)
    inp("wg", (IC, P, HC, PADW), DT)
    inp("wu", (IC, P, HC, PADW), DT)
    inp("wd", (HC, P, IC, PADW), DT)
    outT = nc.declare_dram_parameter("outT", [HIDDEN, T_OWN],
                                     mybir.dt.float32, isOutput=True)
    if os.environ.get("KERNEL_DEBUG_DUMP"):
        for nm, shp in (("dbg_h1T", [HIDDEN, T_FULL]), ("dbg_qT", [NQ * HD, T_OWN]),
                        ("dbg_kT", [HD, T_FULL]), ("dbg_v", [T_FULL, HD]),
                        ("dbg_ctxT", [NQ * HD, T_OWN])):
            dram[nm] = nc.dram_tensor(nm, shp, mybir.dt.float32)
    res2T = nc.dram_tensor("res2T", [HIDDEN, T_OWN], mybir.dt.float32)

    _build_kernel(nc, dram, outT, res2T)
    if not os.environ.get("KERNEL_NO_WAIT_SPLIT"):
        _split_dma_waits(nc)
    _CACHE["nc"] = nc
    return nc


def _split_dma_waits(nc):
    """This walrus encodes at most ONE sync-wait per instruction (the ISA
    EVENTS struct has a single wait slot and this build refuses to split).
    Hoist all waits of multi-wait instructions onto standalone
    event-semaphore instructions on the issuing engine/sequencer, which
    executes them in program order before the original instruction."""
    n = 0
    for f in nc.m.functions:
        for bb in f.blocks:
            out = []
            changed = False
            for inst in bb.instructions:
                si = inst.sync_info
                if si is not None and len(si.on_wait) > 1:
                    for w in si.on_wait:
                        ev = mybir.InstEventSemaphore(
                            name=f"{inst.name}_w{n}", ins=[], outs=[])
                        ev.engine = inst.engine
                        ev.sync_info = mybir.SyncInfo(on_wait=[w],
                                                      on_update=[])
                        out.append(ev)
                        n += 1
                    inst.sync_info = mybir.SyncInfo(
                        on_wait=[], on_update=list(si.on_update))
                    changed = True
                out.append(inst)
            if changed:
                bb.instructions[:] = out
    return n


def _build_kernel(nc, dram, outT, res2T):
    xT_v = dram["xT"][:, :].rearrange("(c p) t -> p c t", p=P)
    res2T_v = res2T[:, :].rearrange("(c p) t -> p c t", p=P)
    outT_v = outT[:, :].rearrange("(c p) t -> p c t", p=P)

    with tile.TileContext(nc) as tc, ExitStack() as top:
        const = top.enter_context(tc.tile_pool(name="const", bufs=1))
        psA = top.enter_context(tc.tile_pool(name="psA", bufs=5, space="PSUM"))
        psB = top.enter_context(tc.tile_pool(name="psB", bufs=2, space="PSUM"))
        psR = top.enter_context(tc.tile_pool(name="psR", bufs=1, space="PSUM"))

        ones_bf = const.tile([P, 1], DT)
        nc.vector.memset(ones_bf, 1.0)
        ones_rf = const.tile([P, 1], mybir.dt.float32, name="ones_rf")
        nc.vector.memset(ones_rf, 1.0)
        ones_r = ones_rf.bitcast(DT_R)
        onesrow_f = const.tile([1, P], mybir.dt.float32, name="onesrow_f")
        nc.vector.memset(onesrow_f, 1.0)
        onesrow_r = onesrow_f.bitcast(DT_R)
        onesrow_bf = const.tile([1, P], DT, name="onesrow_bf")
        nc.vector.memset(onesrow_bf, 1.0)
        strips = top.enter_context(tc.tile_pool(name="strips", bufs=4))

        # ---------------- stage B+C: ada_rms1 (4 slices) pipelined with QKV
        poolBC = tc.alloc_tile_pool(name="poolBC", bufs=1)
        h1T = poolBC.tile([P, HC, T_FULL], DT, name="h1T")
        poolCD = tc.alloc_tile_pool(name="poolCD", bufs=1, side="right")
        qT = poolCD.tile([P, QC, T_OWN], DT, name="qT")
        kT = poolCD.tile([P, KC_HD, T_FULL], DT, name="kT")
        vtok = poolCD.tile([P, HC, HD], DT, name="vtok")

        with tc.tile_pool(name="stBC", bufs=2) as sC:
            cos_f = sC.tile([P, T_FULL], DT, tag="cos", bufs=1, name="cos_f")
            sin_f = sC.tile([P, T_FULL], DT, tag="sin", bufs=1, name="sin_f")
            nc.gpsimd.dma_start(out=cos_f, in_=dram["cosT"][:, :])
            nc.gpsimd.dma_start(out=sin_f, in_=dram["sinT"][:, :])

            rr_tiles = {}

            def rms1_pre(s):
                """load slice s, squares, col sum-of-squares, rsqrt -> rr."""
                sl = slice(s * FD, (s + 1) * FD)
                xt = sC.tile([P, HC, FD], DT, tag="xt", name="xt", bufs=2)
                if s < 2:
                    for q4 in range(4):
                        nc.sync.dma_start(
                            out=xt[:, 4 * q4:4 * (q4 + 1), :],
                            in_=xT_v[:, 4 * q4:4 * (q4 + 1), sl])
                else:
                    nc.gpsimd.dma_start(out=xt, in_=xT_v[:, :, sl])
                ssum = psB.tile([1, FD], mybir.dt.float32, tag="psmall",
                                name="ps_ss")
                for kc in range(HC):
                    sq = sC.tile([P, FD], DT_R, tag="sq", name="sq", bufs=2)
                    nc.scalar.activation(sq, xt[:, kc, :], AF.Square)
                    nc.tensor.matmul(ssum, ones_r, sq,
                                     start=(kc == 0), stop=(kc == HC - 1))
                inv = sC.tile([1, FD], mybir.dt.float32, tag="inv",
                              name="inv", bufs=2)
                nc.vector.reciprocal(inv, ssum)
                # rsqrt(mean_sq) = sqrt(H / ssum); eps is negligible here.
                rr = sC.tile([1, FD], DT_R, tag="rr", name="rr", bufs=2)
                nc.scalar.activation(rr, inv, AF.Sqrt, scale=float(HIDDEN))
                rr_tiles[s] = (xt, rr)

            def rms1_post(s):
                """broadcast rr to all partitions, h1T slice = x * rr."""
                sl = slice(s * FD, (s + 1) * FD)
                xt, rr = rr_tiles.pop(s)
                rrb = psR.tile([P, FD], mybir.dt.float32, tag="bc",
                               name="ps_rrb")
                nc.tensor.matmul(rrb, onesrow_r, rr, start=True, stop=True)
                for kc in range(HC):
                    nc.vector.tensor_mul(h1T[:, kc, sl], xt[:, kc, :], rrb)

            def qproj(nt):
                sl = slice(nt * FD, (nt + 1) * FD)
                for m in range(QC):
                    strip = strips.tile([P, HC, P], DT, tag="w",
                                        name="wq_strip")
                    nc.sync.dma_start(out=strip, in_=dram["wq"][m][:, :, :P])
                    ps = psA.tile([P, FD], mybir.dt.float32, tag="pmm",
                                  name="ps_q")
                    for kc in range(HC):
                        nc.tensor.matmul(ps, strip[:, kc, :], h1T[:, kc, sl],
                                         start=(kc == 0), stop=(kc == HC - 1))
                    nc.vector.tensor_scalar_add(out=qT[:, m, sl], in0=ps,
                                                scalar1=0.0)

            rms1_pre(0)
            rms1_pre(1)
            rms1_post(0)
            rms1_post(1)
            rms1_pre(2)
            rms1_pre(3)
            qproj(0)
            rms1_post(2)
            rms1_post(3)
            qproj(1)

            # K projection over the full batch
            for m in range(KC_HD):
                strip = strips.tile([P, HC, P], DT, tag="w",
                                    name="wk_strip")
                nc.sync.dma_start(out=strip, in_=dram["wk"][m][:, :, :P])
                for nt in range(NT_FULL):
                    sl = slice(nt * FD, (nt + 1) * FD)
                    ps = psA.tile([P, FD], mybir.dt.float32, tag="pmm",
                                  name="ps_k")
                    for kc in range(HC):
                        nc.tensor.matmul(ps, strip[:, kc, :], h1T[:, kc, sl],
                                         start=(kc == 0), stop=(kc == HC - 1))
                    nc.scalar.copy(out=kT[:, m, sl], in_=ps)

            def rope_pair(u, v_, cos_t, sin_t, width, eng, tg):
                t1 = sC.tile([P, width], DT, tag=tg + "1", bufs=1, name="rp1")
                t2 = sC.tile([P, width], DT, tag=tg + "2", bufs=1, name="rp2")
                t3 = sC.tile([P, width], DT, tag=tg + "3", bufs=1, name="rp3")
                t4 = sC.tile([P, width], DT, tag=tg + "4", bufs=1, name="rp4")
                eng.tensor_mul(t1, u, cos_t)
                eng.tensor_mul(t2, u, sin_t)
                eng.tensor_mul(t3, v_, sin_t)
                eng.tensor_mul(t4, v_, cos_t)
                eng.tensor_sub(u, t1, t3)
                eng.tensor_add(v_, t4, t2)

            for hh in range(2):
                ksl = slice(hh * T_OWN, (hh + 1) * T_OWN)
                rope_pair(kT[:, 0, ksl], kT[:, 1, ksl], cos_f[:, ksl],
                          sin_f[:, ksl], T_OWN, nc.vector, "rtv")

            # v token-major: [key-token-in-chunk, key-chunk, hd]
            wv_sb = sC.tile([P, HC, HD], DT, tag="wv", bufs=1, name="wv_sb")
            nc.sync.dma_start(out=wv_sb, in_=dram["wv"][:, :, :].rearrange("c p d -> p c d"))
            for tm in range(T_FULL // P):
                ps = psA.tile([P, HD], mybir.dt.float32, tag="pmm",
                              name="ps_v")
                tsl = slice(tm * P, (tm + 1) * P)
                for kc in range(HC):
                    nc.tensor.matmul(ps, h1T[:, kc, tsl], wv_sb[:, kc, :],
                                     start=(kc == 0), stop=(kc == HC - 1))
                nc.scalar.copy(out=vtok[:, tm, :], in_=ps)

            for h in range(NQ):
                rope_pair(qT[:, 2 * h, :], qT[:, 2 * h + 1, :],
                          cos_f[:, :T_OWN], sin_f[:, :T_OWN], T_OWN,
                          nc.vector, "rtv")

            if "dbg_h1T" in dram:
                dv = dram["dbg_h1T"][:, :].rearrange("(c p) t -> p c t", p=P)
                nc.gpsimd.dma_start(out=dv, in_=h1T)
                dv = dram["dbg_qT"][:, :].rearrange("(c p) t -> p c t", p=P)
                nc.gpsimd.dma_start(out=dv, in_=qT)
                dv = dram["dbg_kT"][:, :].rearrange("(c p) t -> p c t", p=P)
                nc.gpsimd.dma_start(out=dv, in_=kT)
                dv = dram["dbg_v"][:, :].rearrange("(c p) t -> p c t", p=P)
                nc.gpsimd.dma_start(out=dv, in_=vtok)
        poolBC.release()

        # ---------------- stage D: attention, software-pipelined heads
        poolDE = tc.alloc_tile_pool(name="poolDE", bufs=1)
        ctxT = poolDE.tile([P, QC, T_OWN], DT, name="ctxT")
        with tc.tile_pool(name="stD", bufs=2) as sD:

            def scores(h, at):
                for nt in range(NT_OWN):
                    sl = slice(nt * FD, (nt + 1) * FD)
                    for sm in range(T_FULL // P):
                        ps = psA.tile([P, FD], mybir.dt.float32, tag="pmm",
                                      name="ps_sc")
                        for dc in range(KC_HD):
                            nc.tensor.matmul(
                                ps, kT[:, dc, sm * P:(sm + 1) * P],
                                qT[:, 2 * h + dc, sl],
                                start=(dc == 0), stop=(dc == KC_HD - 1))
                        nc.scalar.activation(at[:, sm, sl], ps, AF.Exp,
                                             scale=1.0 / 16.0)

            def colsum_recip(h, at, recs):
                for nt in range(NT_OWN):
                    sl = slice(nt * FD, (nt + 1) * FD)
                    cs = psB.tile([1, FD], mybir.dt.float32, tag="psmall",
                                  name="ps_cs")
                    for kc in range(HC):
                        nc.tensor.matmul(cs, ones_bf, at[:, kc, sl],
                                         start=(kc == 0), stop=(kc == HC - 1))
                    rec = sD.tile([1, FD], mybir.dt.float32, tag=f"rec{nt}",
                                  bufs=2, name="rec")
                    nc.vector.reciprocal(rec, cs)
                    rec_bf = sD.tile([1, FD], DT, tag=f"recbf{nt}",
                                     bufs=2, name="rec_bf")
                    nc.vector.tensor_scalar_add(out=rec_bf, in0=rec,
                                                scalar1=0.0)
                    recs.append(rec_bf)

            def bcast_recb(h, recs, recb):
                for nt in range(NT_OWN):
                    sl = slice(nt * FD, (nt + 1) * FD)
                    rb = psR.tile([P, FD], mybir.dt.float32, tag="bc",
                                  name="ps_recb")
                    nc.tensor.matmul(rb, onesrow_bf, recs[nt],
                                     start=True, stop=True)
                    nc.vector.tensor_scalar_add(out=recb[:, sl], in0=rb,
                                                scalar1=0.0)

            def ctx(h, at, recb):
                for dm in range(KC_HD):
                    for nt in range(NT_OWN):
                        sl = slice(nt * FD, (nt + 1) * FD)
                        ps = psA.tile([P, FD], mybir.dt.float32, tag="pmm",
                                      name="ps_ctx")
                        for kc in range(HC):
                            nc.tensor.matmul(
                                ps, vtok[:, kc, dm * P:(dm + 1) * P],
                                at[:, kc, sl],
                                start=(kc == 0), stop=(kc == HC - 1))
                        nc.vector.tensor_mul(ctxT[:, 2 * h + dm, sl], ps,
                                             recb[:, sl])

            at_tiles = {}
            recb_tiles = {}
            rec_lists = {}

            def new_head(h):
                at_tiles[h] = sD.tile([P, HC, T_OWN], DT, tag="attn", bufs=2,
                                      name="attnT")
                scores(h, at_tiles[h])

            new_head(0)
            for h in range(NQ):
                rec_lists[h] = []
                colsum_recip(h, at_tiles[h], rec_lists[h])
                if h + 1 < NQ:
                    new_head(h + 1)
                recb_tiles[h] = sD.tile([P, T_OWN], DT, tag="recb", bufs=2,
                                        name="recb")
                bcast_recb(h, rec_lists[h], recb_tiles[h])
                ctx(h, at_tiles[h], recb_tiles[h])
                del at_tiles[h], recb_tiles[h], rec_lists[h]

            if "dbg_ctxT" in dram:
                dv = dram["dbg_ctxT"][:, :].rearrange("(c p) t -> p c t", p=P)
                nc.gpsimd.dma_start(out=dv, in_=ctxT)
        poolCD.release()

        # ---------------- stage E: o_proj + residual + ada_rms2, per slice
        poolEF = tc.alloc_tile_pool(name="poolEF", bufs=1, side="right")
        h2T = poolEF.tile([P, HC, T_OWN], DT, name="h2T")
        with tc.tile_pool(name="stE", bufs=2) as sE:
            for nt in range(NT_OWN):
                sl = slice(nt * FD, (nt + 1) * FD)
                res2 = sE.tile([P, HC, FD], mybir.dt.float32, tag="res2",
                               bufs=2, name="res2")
                for m in range(HC):
                    strip = strips.tile([P, QC, P], DT, tag="w",
                                        name="wo_strip")
                    nc.sync.dma_start(out=strip, in_=dram["wo"][m][:, :, :P])
                    ps = psA.tile([P, FD], mybir.dt.float32, tag="pmm",
                                  name="ps_o")
                    for kc in range(QC):
                        nc.tensor.matmul(ps, strip[:, kc, :], ctxT[:, kc, sl],
                                         start=(kc == 0), stop=(kc == QC - 1))
                    xo = sE.tile([P, FD], DT, tag="xo", bufs=4, name="xo")
                    nc.gpsimd.dma_start(out=xo, in_=xT_v[:, m, sl])
                    nc.vector.scalar_tensor_tensor(
                        out=res2[:, m, :], in0=ps, scalar=0.0,
                        in1=xo, op0=ALU.bypass, op1=ALU.add)
                ssum = psB.tile([1, FD], mybir.dt.float32, tag="psmall",
                                name="ps_ss2")
                for kc in range(HC):
                    sq = sE.tile([P, FD], DT_R, tag="sq", name="sq2", bufs=2)
                    nc.scalar.activation(sq, res2[:, kc, :], AF.Square)
                    nc.tensor.matmul(ssum, ones_r, sq,
                                     start=(kc == 0), stop=(kc == HC - 1))
                inv = sE.tile([1, FD], mybir.dt.float32, tag="inv",
                              name="inv2", bufs=2)
                nc.vector.reciprocal(inv, ssum)
                rr = sE.tile([1, FD], DT_R, tag="rr", name="rr2", bufs=2)
                nc.scalar.activation(rr, inv, AF.Sqrt, scale=float(HIDDEN))
                rrb = psR.tile([P, FD], mybir.dt.float32, tag="bc",
                               name="ps_rrb2")
                nc.tensor.matmul(rrb, onesrow_r, rr, start=True, stop=True)
                for kc in range(HC):
                    nc.vector.tensor_mul(h2T[:, kc, sl], res2[:, kc, :], rrb)
                nc.gpsimd.dma_start(out=res2T_v[:, :, sl], in_=res2)
        poolDE.release()

        # ---------------- stage F: SwiGLU MLP + final residual
        with tc.tile_pool(name="stF", bufs=2) as sF:
            for tt in range(NT_OWN):
                sl = slice(tt * FD, (tt + 1) * FD)
                act = sF.tile([P, IC, FD], DT, tag="act", bufs=1, name="act")
                for im in range(IC):
                    gstrip = strips.tile([P, HC, P], DT, tag="w",
                                         name="wg_strip")
                    nc.sync.dma_start(out=gstrip, in_=dram["wg"][im][:, :, :P])
                    ps_g = psA.tile([P, FD], mybir.dt.float32, tag="pmm",
                                    name="ps_g")
                    for kc in range(HC):
                        nc.tensor.matmul(ps_g, gstrip[:, kc, :],
                                         h2T[:, kc, sl],
                                         start=(kc == 0), stop=(kc == HC - 1))
                    sil = sF.tile([P, FD], mybir.dt.float32, tag="sil",
                                  name="sil")
                    nc.scalar.activation(sil, ps_g, AF.Sigmoid)
                    nc.vector.tensor_mul(sil, sil, ps_g)
                    ustrip = strips.tile([P, HC, P], DT, tag="w",
                                          name="wu_strip")
                    nc.sync.dma_start(out=ustrip, in_=dram["wu"][im][:, :, :P])
                    ps_u = psA.tile([P, FD], mybir.dt.float32, tag="pmm",
                                    name="ps_u")
                    for kc in range(HC):
                        nc.tensor.matmul(ps_u, ustrip[:, kc, :],
                                         h2T[:, kc, sl],
                                         start=(kc == 0), stop=(kc == HC - 1))
                    nc.vector.tensor_mul(act[:, im, :], sil, ps_u)
                for dm in range(HC):
                    dstrip = sF.tile([P, IC, P], DT, tag="wd", bufs=3,
                                     name="wd_strip")
                    nc.sync.dma_start(out=dstrip, in_=dram["wd"][dm][:, :, :P])
                    ps_d = psA.tile([P, FD], mybir.dt.float32, tag="pmm",
                                    name="ps_d")
                    for kc in range(IC):
                        nc.tensor.matmul(ps_d, dstrip[:, kc, :],
                                         act[:, kc, :],
                                         start=(kc == 0), stop=(kc == IC - 1))
                    r2c = sF.tile([P, FD], mybir.dt.float32, tag="r2c",
                                  name="r2c", bufs=4)
                    nc.gpsimd.dma_start(out=r2c, in_=res2T_v[:, dm, sl])
                    ot = sF.tile([P, FD], mybir.dt.float32, tag="ot",
                                 name="ot", bufs=4)
                    nc.vector.tensor_add(ot, ps_d, r2c)
                    nc.scalar.dma_start(out=outT_v[:, dm, sl], in_=ot)
        poolEF.release()


def _prep_inputs(x, pos_ids, time_emb, ln1_w, ln1_tw, ln2_w, ln2_tw,
                 Wq, Wk, Wv, Wo, Wg, Wu, Wd):
    """Host-side layout prep. Gains g = w*(1+t@tw.T) are folded into the
    input columns of Wq/Wk/Wv (g1) and Wg/Wu (g2), per batch.
    Returns list of per-core in_maps."""
    x = np.asarray(x, dtype=np.float32)
    time_emb = np.asarray(time_emb, dtype=np.float32)
    g1 = np.asarray(ln1_w) * (1.0 + time_emb @ np.asarray(ln1_tw).T)  # (B,H)
    g2 = np.asarray(ln2_w) * (1.0 + time_emb @ np.asarray(ln2_tw).T)  # (B,H)

    shared = {
        "wo": _strips(np.asarray(Wo).T, QC, HC),
        "wd": _strips(np.asarray(Wd).T, IC, HC),
    }
    per_batch = []
    for b in range(B):
        wq = _strips((np.asarray(Wq) * g1[b][None, :]).T, HC, QC)
        wk = _strips((np.asarray(Wk) * g1[b][None, :]).T, HC, KC_HD)
        wv = np.ascontiguousarray(
            (np.asarray(Wv) * g1[b][None, :]).T.reshape(HC, P, HD)
        ).astype(BF16)
        wg = _strips((np.asarray(Wg) * g2[b][None, :]).T, HC, IC)
        wu = _strips((np.asarray(Wu) * g2[b][None, :]).T, HC, IC)
        per_batch.append({"wq": wq, "wk": wk, "wv": wv, "wg": wg, "wu": wu})

    inv_freq = 1.0 / (ROPE_BASE **
                      (np.arange(0, HD, 2, dtype=np.float64) / HD))
    in_maps = []
    for c in range(N_CORES):
        b, half = c // 2, c % 2
        perm = np.r_[np.arange(half * T_OWN, (half + 1) * T_OWN),
                     np.arange((1 - half) * T_OWN, (2 - half) * T_OWN)]
        xTb = np.ascontiguousarray(np.asarray(x[b]).T[:, perm]).astype(BF16)
        ang = (np.asarray(pos_ids[b])[perm].astype(np.float64)[:, None]
               * inv_freq[None, :])
        m = dict(shared)
        m.update(per_batch[b])
        m["xT"] = xTb
        m["cosT"] = np.ascontiguousarray(np.cos(ang).T).astype(BF16)
        m["sinT"] = np.ascontiguousarray(np.sin(ang).T).astype(BF16)
        in_maps.append(m)
    return in_maps


def kernel(**inputs):
    global LAST_RESULTS
    from concourse.bass_utils import run_bass_kernel_spmd

    nc = build_program()
    in_maps = _prep_inputs(**{k: np.asarray(v) for k, v in inputs.items()})
    trace = bool(int(os.environ.get("KERNEL_TRACE", "0")))
    kw = {}
    if os.environ.get("KERNEL_TMPDIR"):
        os.makedirs(os.environ["KERNEL_TMPDIR"], exist_ok=True)
        kw["tmpdir"] = os.environ["KERNEL_TMPDIR"]
    res = run_bass_kernel_spmd(nc, in_maps, core_ids=list(range(N_CORES)),
                               trace=trace, **kw)
    LAST_RESULTS = res
    out = np.empty((B, S, HIDDEN), dtype=F32)
    for c in range(N_CORES):
        b, half = c // 2, c % 2
        out[b, half * T_OWN:(half + 1) * T_OWN, :] = res.results[c]["outT"].T
    return out
